# revision 26
# baseline (speedup 1.0000x reference)
"""GATv2 (3-layer, 4-head) message-passing kernel for Trainium2, 8-core SPMD.

Strategy (per sharding hint): nodes sharded contiguously across 8 cores;
edges partitioned by destination; per-layer AllGather of the source-side
transform xl = x @ Wl (bf16) so each core can gather arbitrary source rows;
segment softmax / scatter-add stay local per destination shard.

Algebra: with m_e = xl[src_e] + xr[dst_e] (xr = x@Wr + (bl+br)) and
ee = exp(att . leaky(m)), the attention output per destination is
    sum_e ee_e * m_e / denom - xr[dst] + (bl + gat_bias)
so only one row-gather per edge (xl) is needed; the xr side is a
PE segment-broadcast (ST_k^T @ xr_chunk).

Measured-on-HW design notes (5.62ms baseline -> 4.19ms):
- The hard bottleneck is the per-k-tile indirect row gather: the SWDGE
  path costs ~1.4us per instruction (994ns desc-gen + ~310ns sequencer
  gap) regardless of row count/bytes, and only gpsimd can issue it.
  2640 gathers * 1.4us ~= 3.7ms is the floor of this design.
  (Alternatives all fail on this runtime: multi-column offsets return
  nondeterministic garbage, InstDMAGatherAnt dies, ap_gather ucode runs
  ~60us/chunk on the Q7s, 4 SWDGE queues add +210ns to every gather.)
- S (scatter) and ST (broadcast) selection matrices are static per edge
  structure -> precomputed on host, streamed from DRAM on scalar/sync
  hwdge queues (no on-chip is_equal builds, no gpsimd broadcast DMAs).
- per-chunk K is variable (max over cores), and the appended self-loops
  are packed as one dedicated k-tile per chunk whose xl rows are the
  locally-computed aligned block (SBUF xlo_sb) -> no gather for them:
  831 gathers/layer instead of 931.
  NOTE: never DMA out of a persistent SBUF state tile that is rewritten
  per chunk - the WAR against the in-flight DMA serializes the whole
  phase (~0.7ms); DMA from a cycling pool tile and copy separately.
- all K ST-matmuls accumulate into one 5-bank PSUM region; one DVE add
  (+ gathered xl) builds m (replaces 19 tiny adds).
- leaky relu is one scalar-engine Prelu (AP alpha); rstd = rsqrt(var+eps)
  via quake-seed + 2 Newton steps on DVE, keeping every scalar
  activation in one table set (ACT_TABLE reloads: 295 -> 1).
- x (residual, f32), xT (bf16 lhsT), xr (bf16) are SBUF-resident; the
  next layer's node matmuls are interleaved into the edge loop so only
  the AllGather itself (~45us/layer) is exposed.
- Buffer depths are a sharp optimum at bufs=3: deeper prefetch or more
  in-flight gather destinations inflate SWDGE per-instruction time
  (DMA/SBUF contention); shallower starves the consumer chain.
"""

import os
import sys

sys.path.insert(0, "/opt/trn_rl_repo")

import ml_dtypes
import numpy as np

import concourse.bass as bass
import concourse.bacc as bacc
import concourse.tile as tile
from concourse import mybir
from concourse.bass import IndirectOffsetOnAxis

F32 = mybir.dt.float32
BF16 = mybir.dt.bfloat16
I32 = mybir.dt.int32
AF = mybir.ActivationFunctionType
ALU = mybir.AluOpType
AX = mybir.AxisListType

P = 128
NEG_SLOPE = 0.2
LN_EPS = 1e-5
DENOM_EPS = 1e-30
DBG_LAYERS = int(os.environ.get("GAT_LAYERS", "0"))  # 0 = all


class Cfg:
    def __init__(self, N=50000, D=128, H=4, L=3, n_cores=8):
        self.N, self.D, self.H, self.L, self.M = N, D, H, L, n_cores
        self.C = D // H
        assert N % n_cores == 0
        self.shard = N // n_cores
        self.chunks = (self.shard + P - 1) // P


# ----------------------------------------------------------------------------
# Host preprocessing: append self loops, sort by dst, pack per-core chunk
# edge lists, build S / ST selection matrices.
# ----------------------------------------------------------------------------

def preprocess(edge_index, cfg):
    N, M, shard, chunks = cfg.N, cfg.M, cfg.shard, cfg.chunks
    ei = np.asarray(edge_index)
    loops = np.arange(N, dtype=np.int64)
    src = np.concatenate([ei[0].astype(np.int64), loops])
    dst = np.concatenate([ei[1].astype(np.int64), loops])
    isloop = np.concatenate(
        [np.zeros(ei.shape[1], bool), np.ones(N, bool)]
    )
    order = np.argsort(dst, kind="stable")
    src_s, dst_s, loop_s = src[order], dst[order], isloop[order]

    # per-(core, chunk) edge lists
    per_core = []
    cnts = np.zeros((M, chunks), dtype=np.int64)
    for c in range(M):
        lo, hi = np.searchsorted(dst_s, [c * shard, (c + 1) * shard])
        d_loc = dst_s[lo:hi] - c * shard
        s_loc = src_s[lo:hi]
        l_loc = loop_s[lo:hi]
        ch = d_loc // P
        chunk_edges = []
        for t in range(chunks):
            m = (ch == t) & ~l_loc
            chunk_edges.append((s_loc[m], (d_loc[m] - t * P)))
            cnts[c, t] = int(m.sum())
        per_core.append(chunk_edges)

    KG_list = [int(-(-cnts[:, t].max() // P)) for t in range(chunks)]
    K_list = [kg + 1 for kg in KG_list]  # + dedicated self-loop tile
    offs = np.concatenate([[0], np.cumsum(K_list)]).astype(np.int64)
    TOTK = int(offs[-1])

    pre = []
    for c in range(M):
        src_idx = np.zeros((P, TOTK), dtype=np.int32)
        S = np.zeros((P, TOTK * P), dtype=ml_dtypes.bfloat16)
        ST = np.zeros((P, TOTK * P), dtype=ml_dtypes.bfloat16)
        for t in range(chunks):
            sl, dl = per_core[c][t]
            j = np.arange(len(sl))
            p, k = j % P, j // P
            src_idx[p, offs[t] + k] = sl
            S[p, (offs[t] + k) * P + dl] = 1
            ST[dl, (offs[t] + k) * P + p] = 1
            # self-loop tile (k = KG): slot p == dst-local p, identity block
            nt = min(P, shard - t * P)
            ps = np.arange(nt)
            ks = offs[t] + KG_list[t]
            S[ps, ks * P + ps] = 1
            ST[ps, ks * P + ps] = 1
        pre.append({"src_idx": src_idx, "S": S, "ST": ST})
    meta = {"K_list": K_list, "KG_list": KG_list, "offs": offs,
            "TOTK": TOTK}
    return pre, meta


# ----------------------------------------------------------------------------
# Kernel builder
# ----------------------------------------------------------------------------

def build(tc, io, cfg, meta):
    from contextlib import ExitStack

    nc = tc.nc
    D, H, L, C = cfg.D, cfg.H, cfg.L, cfg.C
    shard, chunks = cfg.shard, cfg.chunks
    K_list, offs = meta["K_list"], meta["offs"]
    KG_list = meta["KG_list"]
    TOTK = meta["TOTK"]

    ctx = ExitStack()
    dram = ctx.enter_context(tc.tile_pool(name="drampool", bufs=1, space="DRAM"))
    consts = ctx.enter_context(tc.tile_pool(name="consts", bufs=1))
    state = ctx.enter_context(tc.tile_pool(name="state", bufs=1))
    lconsts = ctx.enter_context(tc.tile_pool(name="lconsts", bufs=2))
    nodep = ctx.enter_context(tc.tile_pool(name="nodep", bufs=3))
    idxp = ctx.enter_context(tc.tile_pool(name="idxp", bufs=3))
    stp = ctx.enter_context(tc.tile_pool(name="stp", bufs=3))
    edgep = ctx.enter_context(tc.tile_pool(name="edgep", bufs=3))
    smallp = ctx.enter_context(tc.tile_pool(name="smallp", bufs=3))
    ps_big = ctx.enter_context(tc.tile_pool(name="ps_big", bufs=1, space="PSUM"))
    ps_o = ctx.enter_context(tc.tile_pool(name="ps_o", bufs=1, space="PSUM"))
    ps_n = ctx.enter_context(tc.tile_pool(name="ps_n", bufs=1, space="PSUM"))
    ps_t = ctx.enter_context(tc.tile_pool(name="ps_t", bufs=1, space="PSUM"))

    KMAX = max(K_list)

    # internal DRAM
    xl_sh = [dram.tile([shard, D], BF16, name=f"xl_sh{l}") for l in range(L)]
    xl_all = [
        dram.tile([cfg.N, D], BF16, name=f"xl_all{l}", addr_space="Shared")
        for l in range(L)
    ]

    # SBUF-resident constants / state
    ident_sb = consts.tile([P, P], F32, name="ident_sb")
    nc.sync.dma_start(out=ident_sb[:], in_=io["ident"][:, :])
    alpha_sb = consts.tile([P, 1], F32, name="alpha_sb")
    nc.vector.memset(alpha_sb[:, :], NEG_SLOPE)
    x_sb = state.tile([P, chunks, D], F32, name="x_sb")
    xT_sb = state.tile([P, chunks * P], BF16, name="xT_sb")
    xr_sb = state.tile([P, chunks, D], BF16, name="xr_sb")
    nc.vector.memset(xr_sb[:, :, :], 0.0)
    xlo_sb = state.tile([P, chunks, D], BF16, name="xlo_sb")
    nc.vector.memset(xlo_sb[:, :, :], 0.0)

    def node_step(l, t, wl, wr, bc):
        """xl = x@Wl -> xl_sh[l]; xr = x@Wr + bc -> xr_sb (bf16)."""
        nt = min(P, shard - t * P)
        lhsT = xT_sb[:, t * P : t * P + nt]
        ps_xl = ps_n.tile([P, D], F32, name="ps_xl", tag="ps_n")
        nc.tensor.matmul(
            out=ps_xl[:nt, :], lhsT=lhsT, rhs=wl[:], start=True, stop=True
        )
        xl_o = nodep.tile([P, D], BF16, name="xl_o")
        nc.scalar.activation(out=xl_o[:nt, :], in_=ps_xl[:nt, :], func=AF.Copy)
        nc.vector.tensor_scalar(
            out=xlo_sb[:nt, t, :], in0=xl_o[:nt, :], scalar1=0.0,
            scalar2=None, op0=ALU.add,
        )
        nc.sync.dma_start(out=xl_sh[l][t * P : t * P + nt, :], in_=xl_o[:nt, :])
        ps_xr = ps_n.tile([P, D], F32, name="ps_xr", tag="ps_n")
        nc.tensor.matmul(
            out=ps_xr[:nt, :], lhsT=lhsT, rhs=wr[:], start=True, stop=True
        )
        nc.vector.tensor_tensor(
            out=xr_sb[:nt, t, :], in0=ps_xr[:nt, :], in1=bc[:nt, :], op=ALU.add
        )

    def allgather(l):
        ins, outs = xl_sh[l][:, :], xl_all[l][:, :]
        nc.gpsimd.collective_compute(
            "AllGather",
            ALU.bypass,
            replica_groups=[list(range(cfg.M))],
            ins=[ins.opt()],
            outs=[outs.opt()],
        )

    def load_node_consts(l):
        wl = lconsts.tile([P, D], BF16, name="wl_sb")
        nc.sync.dma_start(out=wl[:], in_=io["Wl16"][l, :, :])
        wr = lconsts.tile([P, D], BF16, name="wr_sb")
        nc.sync.dma_start(out=wr[:], in_=io["Wr16"][l, :, :])
        bc = lconsts.tile([P, D], F32, name="bc_sb")
        nc.gpsimd.dma_start(out=bc[:], in_=_row_bcast(io["bc"], l, P, D))
        return wl, wr, bc

    # ------------------------------------------------------------------
    # prologue: x -> x_sb, xT; node(0) interleaved; AllGather(0) in halves
    # ------------------------------------------------------------------
    full = (chunks - 1) * P
    nc.sync.dma_start(
        out=x_sb[:, 0 : chunks - 1, :],
        in_=bass.AP(
            tensor=io["x_shard"].tensor, offset=0,
            ap=[[128, P], [P * P, chunks - 1], [1, P]],
        ),
    )
    nc.scalar.dma_start(
        out=x_sb[: shard - full, chunks - 1, :],
        in_=io["x_shard"][full:shard, :],
    )
    nc.scalar.dma_start(out=xT_sb[:, 0:shard], in_=io["xT0"][:, :])
    wl0, wr0, bc0 = load_node_consts(0)
    for t in range(chunks):
        node_step(0, t, wl0, wr0, bc0)
    allgather(0)

    L_eff = DBG_LAYERS if DBG_LAYERS else L
    nconsts = (wl0, wr0, bc0)
    for l in range(L_eff):
        # tail constants for this layer; node constants for the next
        attb_sb = lconsts.tile([P, D], BF16, name="attb_sb")
        nc.gpsimd.dma_start(out=attb_sb[:], in_=_row_bcast(io["attb16"], l, P, D))
        cvec_sb = lconsts.tile([P, D], F32, name="cvec_sb")
        nc.gpsimd.dma_start(out=cvec_sb[:], in_=_row_bcast(io["cvec"], l, P, D))
        gamma_sb = lconsts.tile([P, D], F32, name="gamma_sb")
        nc.gpsimd.dma_start(out=gamma_sb[:], in_=_row_bcast(io["gamma"], l, P, D))
        beta_sb = lconsts.tile([P, D], F32, name="beta_sb")
        nc.gpsimd.dma_start(out=beta_sb[:], in_=_row_bcast(io["beta"], l, P, D))
        if l < L_eff - 1:
            nconsts = load_node_consts(l + 1)

        # --------------------------------------------------------------
        # edge phase (node phase of layer l+1 interleaved per chunk)
        # --------------------------------------------------------------
        def stage_a(ch):
            nt = min(P, shard - ch * P)
            K = K_list[ch]
            KG = KG_list[ch]
            off = int(offs[ch])

            idx_sb = idxp.tile([P, KMAX], I32, name="idx_sb")
            nc.sync.dma_start(out=idx_sb[:, :K], in_=io["src_idx"][:, off : off + K])
            ST_sb = stp.tile([P, KMAX * P], BF16, name="ST_sb")
            nc.sync.dma_start(
                out=ST_sb[:, : K * P], in_=io["ST"][:, off * P : (off + K) * P]
            )
            S_sb = stp.tile([P, KMAX, P], BF16, name="S_sb")
            nc.scalar.dma_start(
                out=S_sb[:, :K, :].rearrange("p k d -> p (k d)"),
                in_=io["S"][:, off * P : (off + K) * P],
            )

            # gather xl rows (the bottleneck: one indirect DMA per k-tile);
            # the last tile holds the self-loops -> local xl, no gather
            g2 = edgep.tile([P, KMAX, D], BF16, name="g2")
            for k in range(KG):
                nc.gpsimd.indirect_dma_start(
                    out=g2[:, k, :],
                    out_offset=None,
                    in_=xl_all[l][:, :],
                    in_offset=IndirectOffsetOnAxis(ap=idx_sb[:, k : k + 1], axis=0),
                )

            # m = (ST_k^T @ xr_chunk) + g2
            m_t = edgep.tile([P, KMAX, D], BF16, name="m_t")
            pm = ps_big.tile([P, KMAX * D], F32, name="pm", tag="pm")
            for k in range(K):
                nc.tensor.matmul(
                    out=pm[:, k * D : (k + 1) * D],
                    lhsT=ST_sb[:, k * P : (k + 1) * P],
                    rhs=xr_sb[:, ch, :],
                    start=True,
                    stop=True,
                )
            nc.vector.tensor_tensor(
                out=m_t[:, :KG, :].rearrange("p k d -> p (k d)"),
                in0=pm[:, : KG * D],
                in1=g2[:, :KG, :].rearrange("p k d -> p (k d)"),
                op=ALU.add,
            )
            nc.vector.tensor_tensor(
                out=m_t[:, KG, :],
                in0=pm[:, KG * D : (KG + 1) * D],
                in1=xlo_sb[:, ch, :],
                op=ALU.add,
            )

            return m_t, S_sb, nt, K, KG

        def stage_b(ch, st):
            m_t, S_sb, nt, K, KG = st
            # leaky relu on the scalar engine (Prelu with AP alpha)
            lk = edgep.tile([P, KMAX, D], BF16, name="lk")
            nc.scalar.activation(
                out=lk[:, :K, :].rearrange("p k d -> p (k d)"),
                in_=m_t[:, :K, :].rearrange("p k d -> p (k d)"),
                func=AF.Prelu,
                alpha=alpha_sb[:, 0:1],
            )

            # attention logits (in place on lk) and exp
            nc.vector.tensor_tensor(
                out=lk[:, :K, :],
                in0=lk[:, :K, :],
                in1=attb_sb[:, :].unsqueeze(1).to_broadcast([P, K, D]),
                op=ALU.mult,
            )
            lg = smallp.tile([P, KMAX, H], F32, name="lg")
            nc.vector.reduce_sum(
                out=lg[:, :K, :],
                in_=lk[:, :K, :].rearrange("p k (h c) -> p k h c", h=H),
                axis=AX.X,
            )
            zee = edgep.tile([P, KMAX, D + H], BF16, name="zee")
            nc.scalar.activation(
                out=zee[:, :K, D : D + H], in_=lg[:, :K, :], func=AF.Exp
            )
            nc.vector.tensor_tensor(
                out=zee[:, :K, 0:D].rearrange("p k (h c) -> p k h c", h=H),
                in0=m_t[:, :K, :].rearrange("p k (h c) -> p k h c", h=H),
                in1=zee[:, :K, D : D + H].unsqueeze(3).to_broadcast([P, K, H, C]),
                op=ALU.mult,
            )

            # segment sums on PE: po[d, 0:D] = sum ee*m ; po[d, D:D+H] = denom
            po = ps_o.tile([P, D + H], F32, name="po", tag="po")
            for k in range(K):
                nc.tensor.matmul(
                    out=po[:, :],
                    lhsT=S_sb[:, k, :],
                    rhs=zee[:, k, :],
                    start=(k == 0),
                    stop=(k == K - 1),
                )

            # normalize, subtract xr, add cvec, residual, LN
            dn = smallp.tile([P, H], F32, name="dn")
            nc.vector.tensor_scalar(
                out=dn[:, :], in0=po[:, D : D + H], scalar1=DENOM_EPS,
                scalar2=None, op0=ALU.add,
            )
            rd = smallp.tile([P, H], F32, name="rd")
            nc.vector.reciprocal(out=rd[:, :], in_=dn[:, :])
            onrm = smallp.tile([P, D], F32, name="onrm")
            nc.vector.tensor_tensor(
                out=onrm[:, :].rearrange("p (h c) -> p h c", h=H),
                in0=po[:, 0:D].rearrange("p (h c) -> p h c", h=H),
                in1=rd[:, :].unsqueeze(2).to_broadcast([P, H, C]),
                op=ALU.mult,
            )
            t1 = smallp.tile([P, D], F32, name="t1")
            nc.vector.tensor_tensor(
                out=t1[:nt, :], in0=onrm[:nt, :], in1=xr_sb[:nt, ch, :],
                op=ALU.subtract,
            )
            t2 = smallp.tile([P, D], F32, name="t2")
            nc.vector.tensor_tensor(
                out=t2[:nt, :], in0=t1[:nt, :], in1=cvec_sb[:nt, :], op=ALU.add
            )
            t3 = smallp.tile([P, D], F32, name="t3")
            nc.vector.tensor_tensor(
                out=t3[:nt, :], in0=t2[:nt, :], in1=x_sb[:nt, ch, :], op=ALU.add
            )

            st6 = smallp.tile([P, 6], F32, name="st6")
            nc.vector.bn_stats(out=st6[:nt, :], in_=t3[:nt, :])
            mv = smallp.tile([P, 2], F32, name="mv")
            nc.vector.bn_aggr(out=mv[:nt, :], in_=st6[:nt, :])
            # rstd = rsqrt(var + eps) on DVE: quake seed + 2 Newton steps
            # (keeps the scalar engine on a single act table set)
            veps = smallp.tile([P, 1], F32, name="veps")
            nc.vector.tensor_scalar(
                out=veps[:nt, :], in0=mv[:nt, 1:2], scalar1=LN_EPS, scalar2=None,
                op0=ALU.add,
            )
            seedi = smallp.tile([P, 1], I32, name="seedi")
            nc.vector.tensor_scalar(
                out=seedi[:nt, :], in0=veps[:nt, :].bitcast(I32), scalar1=1,
                scalar2=None, op0=ALU.logical_shift_right,
            )
            seedf = smallp.tile([P, 1], I32, name="seedf")
            nc.vector.tensor_scalar(
                out=seedf[:nt, :], in0=seedi[:nt, :], scalar1=0x5F3759DF,
                scalar2=-1, op0=ALU.subtract, op1=ALU.mult,
            )
            hhalf = smallp.tile([P, 1], F32, name="hhalf")
            nc.vector.tensor_scalar(
                out=hhalf[:nt, :], in0=veps[:nt, :], scalar1=0.5, scalar2=None,
                op0=ALU.mult,
            )
            y0 = seedf[:nt, :].bitcast(F32)
            nta = smallp.tile([P, 1], F32, name="nta")
            nc.vector.tensor_tensor(out=nta[:nt, :], in0=y0, in1=y0, op=ALU.mult)
            ntb = smallp.tile([P, 1], F32, name="ntb")
            nc.vector.tensor_scalar(
                out=ntb[:nt, :], in0=nta[:nt, :], scalar1=hhalf[:nt, :],
                scalar2=1.5, op0=ALU.mult, op1=ALU.subtract,
            )
            y1n = smallp.tile([P, 1], F32, name="y1n")
            nc.vector.tensor_tensor(out=y1n[:nt, :], in0=ntb[:nt, :], in1=y0, op=ALU.mult)
            nc.vector.tensor_tensor(
                out=nta[:nt, :], in0=y1n[:nt, :], in1=y1n[:nt, :], op=ALU.mult
            )
            nc.vector.tensor_scalar(
                out=ntb[:nt, :], in0=nta[:nt, :], scalar1=hhalf[:nt, :],
                scalar2=1.5, op0=ALU.mult, op1=ALU.subtract,
            )
            rstd = smallp.tile([P, 1], F32, name="rstd")
            nc.vector.tensor_tensor(
                out=rstd[:nt, :], in0=ntb[:nt, :], in1=y1n[:nt, :], op=ALU.mult
            )

            y1 = smallp.tile([P, D], F32, name="y1")
            nc.vector.tensor_scalar(
                out=y1[:nt, :], in0=t3[:nt, :], scalar1=mv[:nt, 0:1],
                scalar2=rstd[:nt, :], op0=ALU.subtract, op1=ALU.mult,
            )
            y2 = smallp.tile([P, D], F32, name="y2")
            nc.vector.tensor_tensor(
                out=y2[:nt, :], in0=y1[:nt, :], in1=gamma_sb[:nt, :], op=ALU.mult
            )
            y3 = smallp.tile([P, D], F32, name="y3")
            nc.vector.tensor_tensor(
                out=y3[:nt, :], in0=y2[:nt, :], in1=beta_sb[:nt, :], op=ALU.add
            )

            if l < L_eff - 1:
                nc.scalar.activation(
                    out=x_sb[:nt, ch, :], in_=y3[:nt, :], func=AF.Relu
                )
                psT2 = ps_t.tile([P, P], F32, name="psT2", tag="psT")
                nc.tensor.transpose(
                    out=psT2[:, :nt], in_=x_sb[:nt, ch, :],
                    identity=ident_sb[:nt, :nt],
                )
                nc.scalar.activation(
                    out=xT_sb[:, ch * P : ch * P + nt], in_=psT2[:, :nt],
                    func=AF.Copy,
                )
                node_step(l + 1, ch, *nconsts)
            else:
                nc.sync.dma_start(
                    out=io["y"][ch * P : ch * P + nt, :], in_=y3[:nt, :]
                )

        for ch in range(chunks):
            stage_b(ch, stage_a(ch))
        if l < L_eff - 1:
            allgather(l + 1)

    ctx.close()


def _row_bcast(ap, row, parts, d):
    """AP reading row `row` of a [R, 1, D] DRAM tensor, replicated across
    `parts` partitions (partition step 0)."""
    flat = ap[row] if ap.ndim == 3 else ap[row : row + 1]
    base = flat.opt()
    return bass.AP(tensor=base.tensor, offset=row * d, ap=[[0, parts], [1, d]])


# ----------------------------------------------------------------------------
# host-side inputs
# ----------------------------------------------------------------------------

def make_host_inputs(inputs, cfg):
    L, D, H, C = cfg.L, cfg.D, cfg.H, cfg.C
    Wl = np.asarray(inputs["Wl"], np.float32)
    Wr = np.asarray(inputs["Wr"], np.float32)
    bl = np.asarray(inputs["bl"], np.float32)
    br = np.asarray(inputs["br"], np.float32)
    att = np.asarray(inputs["att"], np.float32)
    gat_bias = np.asarray(inputs["bias"], np.float32)
    gamma = np.asarray(inputs["gamma"], np.float32)
    beta = np.asarray(inputs["beta"], np.float32)
    return {
        "Wl16": Wl.astype(ml_dtypes.bfloat16),
        "Wr16": Wr.astype(ml_dtypes.bfloat16),
        "attb16": att.reshape(L, 1, H * C).astype(ml_dtypes.bfloat16),
        "bc": (bl + br).reshape(L, 1, D),
        "cvec": (bl + gat_bias).reshape(L, 1, D),
        "gamma": gamma.reshape(L, 1, D),
        "beta": beta.reshape(L, 1, D),
        "ident": np.eye(P, dtype=np.float32),
    }


def make_in_maps(inputs, pre, cfg):
    x = np.asarray(inputs["fine_poi_x"], np.float32)
    shared = make_host_inputs(inputs, cfg)
    in_maps = []
    for c in range(cfg.M):
        m = dict(shared)
        xs = np.ascontiguousarray(x[c * cfg.shard : (c + 1) * cfg.shard])
        m["x_shard"] = xs
        m["xT0"] = np.ascontiguousarray(xs.T).astype(ml_dtypes.bfloat16)
        for k in ("src_idx", "S", "ST"):
            m[k] = pre[c][k]
        in_maps.append(m)
    return in_maps


# ----------------------------------------------------------------------------
# program assembly + execution
# ----------------------------------------------------------------------------

_CACHE = {}


def _build_program(cfg, meta):
    key = (cfg.N, cfg.D, cfg.H, cfg.L, cfg.M, tuple(meta["K_list"]))
    if key in _CACHE:
        return _CACHE[key]
    nc = bacc.Bacc(
        "TRN2", target_bir_lowering=False, debug=False, num_devices=cfg.M
    )
    TOTK = meta["TOTK"]
    io = {}
    io["x_shard"] = nc.dram_tensor(
        "x_shard", [cfg.shard, cfg.D], F32, kind="ExternalInput"
    ).ap()
    io["src_idx"] = nc.dram_tensor(
        "src_idx", [P, TOTK], I32, kind="ExternalInput"
    ).ap()
    io["S"] = nc.dram_tensor("S", [P, TOTK * P], BF16, kind="ExternalInput").ap()
    io["ST"] = nc.dram_tensor("ST", [P, TOTK * P], BF16, kind="ExternalInput").ap()
    io["Wl16"] = nc.dram_tensor(
        "Wl16", [cfg.L, cfg.D, cfg.D], BF16, kind="ExternalInput"
    ).ap()
    io["Wr16"] = nc.dram_tensor(
        "Wr16", [cfg.L, cfg.D, cfg.D], BF16, kind="ExternalInput"
    ).ap()
    io["attb16"] = nc.dram_tensor(
        "attb16", [cfg.L, 1, cfg.D], BF16, kind="ExternalInput"
    ).ap()
    for nm in ["bc", "cvec", "gamma", "beta"]:
        io[nm] = nc.dram_tensor(
            nm, [cfg.L, 1, cfg.D], F32, kind="ExternalInput"
        ).ap()
    io["ident"] = nc.dram_tensor("ident", [P, P], F32, kind="ExternalInput").ap()
    io["xT0"] = nc.dram_tensor(
        "xT0", [P, cfg.shard], BF16, kind="ExternalInput"
    ).ap()
    io["y"] = nc.dram_tensor(
        "y", [cfg.shard, cfg.D], F32, kind="ExternalOutput"
    ).ap()

    with tile.TileContext(nc) as tc:
        build(tc, io, cfg, meta)
    nc.compile()
    _CACHE[key] = nc
    return nc


def kernel(**inputs):
    from concourse import bass_utils

    cfg = Cfg()
    pre, meta = preprocess(inputs["edge_index"], cfg)
    nc = _build_program(cfg, meta)
    in_maps = make_in_maps(inputs, pre, cfg)
    res = bass_utils.run_bass_kernel_spmd(nc, in_maps, core_ids=list(range(cfg.M)))
    out = np.concatenate([res.results[c]["y"] for c in range(cfg.M)], axis=0)
    return out.astype(np.float32)


# revision 27
# speedup vs baseline: 1.0065x; 1.0065x over previous
"""GATv2 (3-layer, 4-head) message-passing kernel for Trainium2, 8-core SPMD.

Strategy (per sharding hint): nodes sharded contiguously across 8 cores;
edges partitioned by destination; per-layer AllGather of the source-side
transform xl = x @ Wl (bf16) so each core can gather arbitrary source rows;
segment softmax / scatter-add stay local per destination shard.

Algebra: with m_e = xl[src_e] + xr[dst_e] (xr = x@Wr + (bl+br)) and
ee = exp(att . leaky(m)), the attention output per destination is
    sum_e ee_e * m_e / denom - xr[dst] + (bl + gat_bias)
so only one row-gather per edge (xl) is needed; the xr side is a
PE segment-broadcast (ST_k^T @ xr_chunk).

Measured-on-HW design notes (5.62ms baseline -> 4.19ms):
- The hard bottleneck is the per-k-tile indirect row gather: the SWDGE
  path costs ~1.4us per instruction (994ns desc-gen + ~310ns sequencer
  gap) regardless of row count/bytes, and only gpsimd can issue it.
  2640 gathers * 1.4us ~= 3.7ms is the floor of this design.
  (Alternatives all fail on this runtime: multi-column offsets return
  nondeterministic garbage, InstDMAGatherAnt dies, ap_gather ucode runs
  ~60us/chunk on the Q7s, 4 SWDGE queues add +210ns to every gather.)
- S (scatter) and ST (broadcast) selection matrices are static per edge
  structure -> precomputed on host, streamed from DRAM on scalar/sync
  hwdge queues (no on-chip is_equal builds, no gpsimd broadcast DMAs).
- per-chunk K is variable (max over cores), and the appended self-loops
  are packed as one dedicated k-tile per chunk whose xl rows are the
  locally-computed aligned block (SBUF xlo_sb) -> no gather for them:
  831 gathers/layer instead of 931.
  NOTE: never DMA out of a persistent SBUF state tile that is rewritten
  per chunk - the WAR against the in-flight DMA serializes the whole
  phase (~0.7ms); DMA from a cycling pool tile and copy separately.
- all K ST-matmuls accumulate into one 5-bank PSUM region; one DVE add
  (+ gathered xl) builds m (replaces 19 tiny adds).
- leaky relu is one scalar-engine Prelu (AP alpha); rstd = rsqrt(var+eps)
  via quake-seed + 2 Newton steps on DVE, keeping every scalar
  activation in one table set (ACT_TABLE reloads: 295 -> 1).
- x (residual, f32), xT (bf16 lhsT), xr (bf16) are SBUF-resident; the
  next layer's node matmuls are interleaved into the edge loop so only
  the AllGather itself (~45us/layer) is exposed.
- Buffer depths are a sharp optimum at bufs=3: deeper prefetch or more
  in-flight gather destinations inflate SWDGE per-instruction time
  (DMA/SBUF contention); shallower starves the consumer chain.
"""

import os
import sys

sys.path.insert(0, "/opt/trn_rl_repo")

import ml_dtypes
import numpy as np

import concourse.bass as bass
import concourse.bacc as bacc
import concourse.tile as tile
from concourse import mybir
from concourse.bass import IndirectOffsetOnAxis

F32 = mybir.dt.float32
BF16 = mybir.dt.bfloat16
I32 = mybir.dt.int32
AF = mybir.ActivationFunctionType
ALU = mybir.AluOpType
AX = mybir.AxisListType

P = 128
NEG_SLOPE = 0.2
LN_EPS = 1e-5
DENOM_EPS = 1e-30
DBG_LAYERS = int(os.environ.get("GAT_LAYERS", "0"))  # 0 = all


class Cfg:
    def __init__(self, N=50000, D=128, H=4, L=3, n_cores=8):
        self.N, self.D, self.H, self.L, self.M = N, D, H, L, n_cores
        self.C = D // H
        assert N % n_cores == 0
        self.shard = N // n_cores
        self.chunks = (self.shard + P - 1) // P


# ----------------------------------------------------------------------------
# Host preprocessing: append self loops, sort by dst, pack per-core chunk
# edge lists, build S / ST selection matrices.
# ----------------------------------------------------------------------------

def preprocess(edge_index, cfg):
    N, M, shard, chunks = cfg.N, cfg.M, cfg.shard, cfg.chunks
    ei = np.asarray(edge_index)
    loops = np.arange(N, dtype=np.int64)
    src = np.concatenate([ei[0].astype(np.int64), loops])
    dst = np.concatenate([ei[1].astype(np.int64), loops])
    isloop = np.concatenate(
        [np.zeros(ei.shape[1], bool), np.ones(N, bool)]
    )
    order = np.argsort(dst, kind="stable")
    src_s, dst_s, loop_s = src[order], dst[order], isloop[order]

    # per-(core, chunk) edge lists
    per_core = []
    cnts = np.zeros((M, chunks), dtype=np.int64)
    for c in range(M):
        lo, hi = np.searchsorted(dst_s, [c * shard, (c + 1) * shard])
        d_loc = dst_s[lo:hi] - c * shard
        s_loc = src_s[lo:hi]
        l_loc = loop_s[lo:hi]
        ch = d_loc // P
        chunk_edges = []
        for t in range(chunks):
            m = (ch == t) & ~l_loc
            chunk_edges.append((s_loc[m], (d_loc[m] - t * P)))
            cnts[c, t] = int(m.sum())
        per_core.append(chunk_edges)

    KG_list = [int(-(-cnts[:, t].max() // P)) for t in range(chunks)]
    K_list = [kg + 1 for kg in KG_list]  # + dedicated self-loop tile
    offs = np.concatenate([[0], np.cumsum(K_list)]).astype(np.int64)
    TOTK = int(offs[-1])

    pre = []
    for c in range(M):
        src_idx = np.zeros((P, TOTK), dtype=np.int32)
        S = np.zeros((P, TOTK * P), dtype=ml_dtypes.bfloat16)
        ST = np.zeros((P, TOTK * P), dtype=ml_dtypes.bfloat16)
        for t in range(chunks):
            sl, dl = per_core[c][t]
            j = np.arange(len(sl))
            p, k = j % P, j // P
            src_idx[p, offs[t] + k] = sl
            S[p, (offs[t] + k) * P + dl] = 1
            ST[dl, (offs[t] + k) * P + p] = 1
            # self-loop tile (k = KG): slot p == dst-local p, identity block
            nt = min(P, shard - t * P)
            ps = np.arange(nt)
            ks = offs[t] + KG_list[t]
            S[ps, ks * P + ps] = 1
            ST[ps, ks * P + ps] = 1
        pre.append({"src_idx": src_idx, "S": S, "ST": ST})
    meta = {"K_list": K_list, "KG_list": KG_list, "offs": offs,
            "TOTK": TOTK}
    return pre, meta


# ----------------------------------------------------------------------------
# Kernel builder
# ----------------------------------------------------------------------------

def build(tc, io, cfg, meta):
    from contextlib import ExitStack

    nc = tc.nc
    D, H, L, C = cfg.D, cfg.H, cfg.L, cfg.C
    shard, chunks = cfg.shard, cfg.chunks
    K_list, offs = meta["K_list"], meta["offs"]
    KG_list = meta["KG_list"]
    TOTK = meta["TOTK"]

    ctx = ExitStack()
    dram = ctx.enter_context(tc.tile_pool(name="drampool", bufs=1, space="DRAM"))
    consts = ctx.enter_context(tc.tile_pool(name="consts", bufs=1))
    state = ctx.enter_context(tc.tile_pool(name="state", bufs=1))
    lconsts = ctx.enter_context(tc.tile_pool(name="lconsts", bufs=2))
    nodep = ctx.enter_context(tc.tile_pool(name="nodep", bufs=3))
    idxp = ctx.enter_context(tc.tile_pool(name="idxp", bufs=3))
    stp = ctx.enter_context(tc.tile_pool(name="stp", bufs=3))
    edgep = ctx.enter_context(tc.tile_pool(name="edgep", bufs=3))
    smallp = ctx.enter_context(tc.tile_pool(name="smallp", bufs=3))
    ps_big = ctx.enter_context(tc.tile_pool(name="ps_big", bufs=1, space="PSUM"))
    ps_o = ctx.enter_context(tc.tile_pool(name="ps_o", bufs=1, space="PSUM"))
    ps_n = ctx.enter_context(tc.tile_pool(name="ps_n", bufs=1, space="PSUM"))
    ps_t = ctx.enter_context(tc.tile_pool(name="ps_t", bufs=1, space="PSUM"))

    KMAX = max(K_list)

    # internal DRAM
    xl_sh = [dram.tile([shard, D], BF16, name=f"xl_sh{l}") for l in range(L)]
    xl_all = [
        dram.tile([cfg.N, D], BF16, name=f"xl_all{l}", addr_space="Shared")
        for l in range(L)
    ]

    # SBUF-resident constants / state
    ident_sb = consts.tile([P, P], F32, name="ident_sb")
    nc.sync.dma_start(out=ident_sb[:], in_=io["ident"][:, :])
    alpha_sb = consts.tile([P, 1], F32, name="alpha_sb")
    nc.vector.memset(alpha_sb[:, :], NEG_SLOPE)
    x_sb = state.tile([P, chunks, D], F32, name="x_sb")
    xT_sb = state.tile([P, chunks * P], BF16, name="xT_sb")
    xr_sb = state.tile([P, chunks, D], BF16, name="xr_sb")
    nc.vector.memset(xr_sb[:, :, :], 0.0)
    xlo_sb = state.tile([P, chunks, D], BF16, name="xlo_sb")
    nc.vector.memset(xlo_sb[:, :, :], 0.0)

    def node_step(l, t, wl, wr, bc):
        """xl = x@Wl -> xl_sh[l]; xr = x@Wr + bc -> xr_sb (bf16)."""
        nt = min(P, shard - t * P)
        lhsT = xT_sb[:, t * P : t * P + nt]
        ps_xl = ps_n.tile([P, D], F32, name="ps_xl", tag="ps_n")
        nc.tensor.matmul(
            out=ps_xl[:nt, :], lhsT=lhsT, rhs=wl[:], start=True, stop=True
        )
        xl_o = nodep.tile([P, D], BF16, name="xl_o")
        nc.scalar.activation(out=xl_o[:nt, :], in_=ps_xl[:nt, :], func=AF.Copy)
        nc.vector.tensor_scalar(
            out=xlo_sb[:nt, t, :], in0=xl_o[:nt, :], scalar1=0.0,
            scalar2=None, op0=ALU.add,
        )
        nc.sync.dma_start(out=xl_sh[l][t * P : t * P + nt, :], in_=xl_o[:nt, :])
        ps_xr = ps_n.tile([P, D], F32, name="ps_xr", tag="ps_n")
        nc.tensor.matmul(
            out=ps_xr[:nt, :], lhsT=lhsT, rhs=wr[:], start=True, stop=True
        )
        nc.vector.tensor_tensor(
            out=xr_sb[:nt, t, :], in0=ps_xr[:nt, :], in1=bc[:nt, :], op=ALU.add
        )

    def allgather(l):
        ins, outs = xl_sh[l][:, :], xl_all[l][:, :]
        nc.gpsimd.collective_compute(
            "AllGather",
            ALU.bypass,
            replica_groups=[list(range(cfg.M))],
            ins=[ins.opt()],
            outs=[outs.opt()],
        )

    def load_node_consts(l):
        wl = lconsts.tile([P, D], BF16, name="wl_sb")
        nc.sync.dma_start(out=wl[:], in_=io["Wl16"][l, :, :])
        wr = lconsts.tile([P, D], BF16, name="wr_sb")
        nc.sync.dma_start(out=wr[:], in_=io["Wr16"][l, :, :])
        bc = lconsts.tile([P, D], F32, name="bc_sb")
        nc.gpsimd.dma_start(out=bc[:], in_=_row_bcast(io["bc"], l, P, D))
        return wl, wr, bc

    # ------------------------------------------------------------------
    # prologue: x -> x_sb, xT; node(0) interleaved; AllGather(0) in halves
    # ------------------------------------------------------------------
    full = (chunks - 1) * P
    nc.sync.dma_start(
        out=x_sb[:, 0 : chunks - 1, :],
        in_=bass.AP(
            tensor=io["x_shard"].tensor, offset=0,
            ap=[[128, P], [P * P, chunks - 1], [1, P]],
        ),
    )
    nc.scalar.dma_start(
        out=x_sb[: shard - full, chunks - 1, :],
        in_=io["x_shard"][full:shard, :],
    )
    wl0, wr0, bc0 = load_node_consts(0)
    for t in range(chunks):
        nt = min(P, shard - t * P)
        psT = ps_t.tile([P, P], F32, name="psT", tag="psT")
        nc.tensor.transpose(
            out=psT[:, :nt], in_=x_sb[:nt, t, :], identity=ident_sb[:nt, :nt]
        )
        nc.scalar.activation(
            out=xT_sb[:, t * P : t * P + nt], in_=psT[:, :nt], func=AF.Copy
        )
        node_step(0, t, wl0, wr0, bc0)
    allgather(0)

    L_eff = DBG_LAYERS if DBG_LAYERS else L
    nconsts = (wl0, wr0, bc0)
    for l in range(L_eff):
        # tail constants for this layer; node constants for the next
        attb_sb = lconsts.tile([P, D], BF16, name="attb_sb")
        nc.gpsimd.dma_start(out=attb_sb[:], in_=_row_bcast(io["attb16"], l, P, D))
        cvec_sb = lconsts.tile([P, D], F32, name="cvec_sb")
        nc.gpsimd.dma_start(out=cvec_sb[:], in_=_row_bcast(io["cvec"], l, P, D))
        gamma_sb = lconsts.tile([P, D], F32, name="gamma_sb")
        nc.gpsimd.dma_start(out=gamma_sb[:], in_=_row_bcast(io["gamma"], l, P, D))
        beta_sb = lconsts.tile([P, D], F32, name="beta_sb")
        nc.gpsimd.dma_start(out=beta_sb[:], in_=_row_bcast(io["beta"], l, P, D))
        if l < L_eff - 1:
            nconsts = load_node_consts(l + 1)

        # --------------------------------------------------------------
        # edge phase (node phase of layer l+1 interleaved per chunk)
        # --------------------------------------------------------------
        def stage_a(ch):
            nt = min(P, shard - ch * P)
            K = K_list[ch]
            KG = KG_list[ch]
            off = int(offs[ch])

            idx_sb = idxp.tile([P, KMAX], I32, name="idx_sb")
            nc.sync.dma_start(out=idx_sb[:, :K], in_=io["src_idx"][:, off : off + K])
            ST_sb = stp.tile([P, KMAX * P], BF16, name="ST_sb")
            nc.sync.dma_start(
                out=ST_sb[:, : K * P], in_=io["ST"][:, off * P : (off + K) * P]
            )
            S_sb = stp.tile([P, KMAX, P], BF16, name="S_sb")
            nc.scalar.dma_start(
                out=S_sb[:, :K, :].rearrange("p k d -> p (k d)"),
                in_=io["S"][:, off * P : (off + K) * P],
            )

            # gather xl rows (the bottleneck: one indirect DMA per k-tile);
            # the last tile holds the self-loops -> local xl, no gather
            g2 = edgep.tile([P, KMAX, D], BF16, name="g2")
            for k in range(KG):
                nc.gpsimd.indirect_dma_start(
                    out=g2[:, k, :],
                    out_offset=None,
                    in_=xl_all[l][:, :],
                    in_offset=IndirectOffsetOnAxis(ap=idx_sb[:, k : k + 1], axis=0),
                )

            # m = (ST_k^T @ xr_chunk) + g2
            m_t = edgep.tile([P, KMAX, D], BF16, name="m_t")
            pm = ps_big.tile([P, KMAX * D], F32, name="pm", tag="pm")
            for k in range(K):
                nc.tensor.matmul(
                    out=pm[:, k * D : (k + 1) * D],
                    lhsT=ST_sb[:, k * P : (k + 1) * P],
                    rhs=xr_sb[:, ch, :],
                    start=True,
                    stop=True,
                )
            nc.vector.tensor_tensor(
                out=m_t[:, :KG, :].rearrange("p k d -> p (k d)"),
                in0=pm[:, : KG * D],
                in1=g2[:, :KG, :].rearrange("p k d -> p (k d)"),
                op=ALU.add,
            )
            nc.vector.tensor_tensor(
                out=m_t[:, KG, :],
                in0=pm[:, KG * D : (KG + 1) * D],
                in1=xlo_sb[:, ch, :],
                op=ALU.add,
            )

            return m_t, S_sb, nt, K, KG

        def stage_b(ch, st):
            m_t, S_sb, nt, K, KG = st
            # leaky relu on the scalar engine (Prelu with AP alpha)
            lk = edgep.tile([P, KMAX, D], BF16, name="lk")
            nc.scalar.activation(
                out=lk[:, :K, :].rearrange("p k d -> p (k d)"),
                in_=m_t[:, :K, :].rearrange("p k d -> p (k d)"),
                func=AF.Prelu,
                alpha=alpha_sb[:, 0:1],
            )

            # attention logits (in place on lk) and exp
            nc.vector.tensor_tensor(
                out=lk[:, :K, :],
                in0=lk[:, :K, :],
                in1=attb_sb[:, :].unsqueeze(1).to_broadcast([P, K, D]),
                op=ALU.mult,
            )
            lg = smallp.tile([P, KMAX, H], F32, name="lg")
            nc.vector.reduce_sum(
                out=lg[:, :K, :],
                in_=lk[:, :K, :].rearrange("p k (h c) -> p k h c", h=H),
                axis=AX.X,
            )
            zee = edgep.tile([P, KMAX, D + H], BF16, name="zee")
            nc.scalar.activation(
                out=zee[:, :K, D : D + H], in_=lg[:, :K, :], func=AF.Exp
            )
            nc.vector.tensor_tensor(
                out=zee[:, :K, 0:D].rearrange("p k (h c) -> p k h c", h=H),
                in0=m_t[:, :K, :].rearrange("p k (h c) -> p k h c", h=H),
                in1=zee[:, :K, D : D + H].unsqueeze(3).to_broadcast([P, K, H, C]),
                op=ALU.mult,
            )

            # segment sums on PE: po[d, 0:D] = sum ee*m ; po[d, D:D+H] = denom
            po = ps_o.tile([P, D + H], F32, name="po", tag="po")
            for k in range(K):
                nc.tensor.matmul(
                    out=po[:, :],
                    lhsT=S_sb[:, k, :],
                    rhs=zee[:, k, :],
                    start=(k == 0),
                    stop=(k == K - 1),
                )

            # normalize, subtract xr, add cvec, residual, LN
            dn = smallp.tile([P, H], F32, name="dn")
            nc.vector.tensor_scalar(
                out=dn[:, :], in0=po[:, D : D + H], scalar1=DENOM_EPS,
                scalar2=None, op0=ALU.add,
            )
            rd = smallp.tile([P, H], F32, name="rd")
            nc.vector.reciprocal(out=rd[:, :], in_=dn[:, :])
            onrm = smallp.tile([P, D], F32, name="onrm")
            nc.vector.tensor_tensor(
                out=onrm[:, :].rearrange("p (h c) -> p h c", h=H),
                in0=po[:, 0:D].rearrange("p (h c) -> p h c", h=H),
                in1=rd[:, :].unsqueeze(2).to_broadcast([P, H, C]),
                op=ALU.mult,
            )
            t1 = smallp.tile([P, D], F32, name="t1")
            nc.vector.tensor_tensor(
                out=t1[:nt, :], in0=onrm[:nt, :], in1=xr_sb[:nt, ch, :],
                op=ALU.subtract,
            )
            t2 = smallp.tile([P, D], F32, name="t2")
            nc.vector.tensor_tensor(
                out=t2[:nt, :], in0=t1[:nt, :], in1=cvec_sb[:nt, :], op=ALU.add
            )
            t3 = smallp.tile([P, D], F32, name="t3")
            nc.vector.tensor_tensor(
                out=t3[:nt, :], in0=t2[:nt, :], in1=x_sb[:nt, ch, :], op=ALU.add
            )

            st6 = smallp.tile([P, 6], F32, name="st6")
            nc.vector.bn_stats(out=st6[:nt, :], in_=t3[:nt, :])
            mv = smallp.tile([P, 2], F32, name="mv")
            nc.vector.bn_aggr(out=mv[:nt, :], in_=st6[:nt, :])
            # rstd = rsqrt(var + eps) on DVE: quake seed + 2 Newton steps
            # (keeps the scalar engine on a single act table set)
            veps = smallp.tile([P, 1], F32, name="veps")
            nc.vector.tensor_scalar(
                out=veps[:nt, :], in0=mv[:nt, 1:2], scalar1=LN_EPS, scalar2=None,
                op0=ALU.add,
            )
            seedi = smallp.tile([P, 1], I32, name="seedi")
            nc.vector.tensor_scalar(
                out=seedi[:nt, :], in0=veps[:nt, :].bitcast(I32), scalar1=1,
                scalar2=None, op0=ALU.logical_shift_right,
            )
            seedf = smallp.tile([P, 1], I32, name="seedf")
            nc.vector.tensor_scalar(
                out=seedf[:nt, :], in0=seedi[:nt, :], scalar1=0x5F3759DF,
                scalar2=-1, op0=ALU.subtract, op1=ALU.mult,
            )
            hhalf = smallp.tile([P, 1], F32, name="hhalf")
            nc.vector.tensor_scalar(
                out=hhalf[:nt, :], in0=veps[:nt, :], scalar1=0.5, scalar2=None,
                op0=ALU.mult,
            )
            y0 = seedf[:nt, :].bitcast(F32)
            nta = smallp.tile([P, 1], F32, name="nta")
            nc.vector.tensor_tensor(out=nta[:nt, :], in0=y0, in1=y0, op=ALU.mult)
            ntb = smallp.tile([P, 1], F32, name="ntb")
            nc.vector.tensor_scalar(
                out=ntb[:nt, :], in0=nta[:nt, :], scalar1=hhalf[:nt, :],
                scalar2=1.5, op0=ALU.mult, op1=ALU.subtract,
            )
            y1n = smallp.tile([P, 1], F32, name="y1n")
            nc.vector.tensor_tensor(out=y1n[:nt, :], in0=ntb[:nt, :], in1=y0, op=ALU.mult)
            nc.vector.tensor_tensor(
                out=nta[:nt, :], in0=y1n[:nt, :], in1=y1n[:nt, :], op=ALU.mult
            )
            nc.vector.tensor_scalar(
                out=ntb[:nt, :], in0=nta[:nt, :], scalar1=hhalf[:nt, :],
                scalar2=1.5, op0=ALU.mult, op1=ALU.subtract,
            )
            rstd = smallp.tile([P, 1], F32, name="rstd")
            nc.vector.tensor_tensor(
                out=rstd[:nt, :], in0=ntb[:nt, :], in1=y1n[:nt, :], op=ALU.mult
            )

            y1 = smallp.tile([P, D], F32, name="y1")
            nc.vector.tensor_scalar(
                out=y1[:nt, :], in0=t3[:nt, :], scalar1=mv[:nt, 0:1],
                scalar2=rstd[:nt, :], op0=ALU.subtract, op1=ALU.mult,
            )
            y2 = smallp.tile([P, D], F32, name="y2")
            nc.vector.tensor_tensor(
                out=y2[:nt, :], in0=y1[:nt, :], in1=gamma_sb[:nt, :], op=ALU.mult
            )
            y3 = smallp.tile([P, D], F32, name="y3")
            nc.vector.tensor_tensor(
                out=y3[:nt, :], in0=y2[:nt, :], in1=beta_sb[:nt, :], op=ALU.add
            )

            if l < L_eff - 1:
                nc.scalar.activation(
                    out=x_sb[:nt, ch, :], in_=y3[:nt, :], func=AF.Relu
                )
                psT2 = ps_t.tile([P, P], F32, name="psT2", tag="psT")
                nc.tensor.transpose(
                    out=psT2[:, :nt], in_=x_sb[:nt, ch, :],
                    identity=ident_sb[:nt, :nt],
                )
                nc.scalar.activation(
                    out=xT_sb[:, ch * P : ch * P + nt], in_=psT2[:, :nt],
                    func=AF.Copy,
                )
                node_step(l + 1, ch, *nconsts)
            else:
                nc.sync.dma_start(
                    out=io["y"][ch * P : ch * P + nt, :], in_=y3[:nt, :]
                )

        for ch in range(chunks):
            stage_b(ch, stage_a(ch))
        if l < L_eff - 1:
            allgather(l + 1)

    ctx.close()


def _row_bcast(ap, row, parts, d):
    """AP reading row `row` of a [R, 1, D] DRAM tensor, replicated across
    `parts` partitions (partition step 0)."""
    flat = ap[row] if ap.ndim == 3 else ap[row : row + 1]
    base = flat.opt()
    return bass.AP(tensor=base.tensor, offset=row * d, ap=[[0, parts], [1, d]])


# ----------------------------------------------------------------------------
# host-side inputs
# ----------------------------------------------------------------------------

def make_host_inputs(inputs, cfg):
    L, D, H, C = cfg.L, cfg.D, cfg.H, cfg.C
    Wl = np.asarray(inputs["Wl"], np.float32)
    Wr = np.asarray(inputs["Wr"], np.float32)
    bl = np.asarray(inputs["bl"], np.float32)
    br = np.asarray(inputs["br"], np.float32)
    att = np.asarray(inputs["att"], np.float32)
    gat_bias = np.asarray(inputs["bias"], np.float32)
    gamma = np.asarray(inputs["gamma"], np.float32)
    beta = np.asarray(inputs["beta"], np.float32)
    return {
        "Wl16": Wl.astype(ml_dtypes.bfloat16),
        "Wr16": Wr.astype(ml_dtypes.bfloat16),
        "attb16": att.reshape(L, 1, H * C).astype(ml_dtypes.bfloat16),
        "bc": (bl + br).reshape(L, 1, D),
        "cvec": (bl + gat_bias).reshape(L, 1, D),
        "gamma": gamma.reshape(L, 1, D),
        "beta": beta.reshape(L, 1, D),
        "ident": np.eye(P, dtype=np.float32),
    }


def make_in_maps(inputs, pre, cfg):
    x = np.asarray(inputs["fine_poi_x"], np.float32)
    shared = make_host_inputs(inputs, cfg)
    in_maps = []
    for c in range(cfg.M):
        m = dict(shared)
        m["x_shard"] = np.ascontiguousarray(x[c * cfg.shard : (c + 1) * cfg.shard])
        for k in ("src_idx", "S", "ST"):
            m[k] = pre[c][k]
        in_maps.append(m)
    return in_maps


# ----------------------------------------------------------------------------
# program assembly + execution
# ----------------------------------------------------------------------------

_CACHE = {}


def _build_program(cfg, meta):
    key = (cfg.N, cfg.D, cfg.H, cfg.L, cfg.M, tuple(meta["K_list"]))
    if key in _CACHE:
        return _CACHE[key]
    nc = bacc.Bacc(
        "TRN2", target_bir_lowering=False, debug=False, num_devices=cfg.M
    )
    TOTK = meta["TOTK"]
    io = {}
    io["x_shard"] = nc.dram_tensor(
        "x_shard", [cfg.shard, cfg.D], F32, kind="ExternalInput"
    ).ap()
    io["src_idx"] = nc.dram_tensor(
        "src_idx", [P, TOTK], I32, kind="ExternalInput"
    ).ap()
    io["S"] = nc.dram_tensor("S", [P, TOTK * P], BF16, kind="ExternalInput").ap()
    io["ST"] = nc.dram_tensor("ST", [P, TOTK * P], BF16, kind="ExternalInput").ap()
    io["Wl16"] = nc.dram_tensor(
        "Wl16", [cfg.L, cfg.D, cfg.D], BF16, kind="ExternalInput"
    ).ap()
    io["Wr16"] = nc.dram_tensor(
        "Wr16", [cfg.L, cfg.D, cfg.D], BF16, kind="ExternalInput"
    ).ap()
    io["attb16"] = nc.dram_tensor(
        "attb16", [cfg.L, 1, cfg.D], BF16, kind="ExternalInput"
    ).ap()
    for nm in ["bc", "cvec", "gamma", "beta"]:
        io[nm] = nc.dram_tensor(
            nm, [cfg.L, 1, cfg.D], F32, kind="ExternalInput"
        ).ap()
    io["ident"] = nc.dram_tensor("ident", [P, P], F32, kind="ExternalInput").ap()
    io["y"] = nc.dram_tensor(
        "y", [cfg.shard, cfg.D], F32, kind="ExternalOutput"
    ).ap()

    with tile.TileContext(nc) as tc:
        build(tc, io, cfg, meta)
    nc.compile()
    _CACHE[key] = nc
    return nc


def kernel(**inputs):
    from concourse import bass_utils

    cfg = Cfg()
    pre, meta = preprocess(inputs["edge_index"], cfg)
    nc = _build_program(cfg, meta)
    in_maps = make_in_maps(inputs, pre, cfg)
    res = bass_utils.run_bass_kernel_spmd(nc, in_maps, core_ids=list(range(cfg.M)))
    out = np.concatenate([res.results[c]["y"] for c in range(cfg.M)], axis=0)
    return out.astype(np.float32)
